# revision 1
# baseline (speedup 1.0000x reference)
# DenseGATv2Conv Trainium2 kernel (v2).
#
# Math (per batch b):
#   xl = x @ W_l + b_l ; xr = x @ W_r + b_r            [N, H*C]
#   alpha[i,j,h] = sum_c att[h,c] * leaky_relu(xl[j,hc] + xr[i,hc], 0.2)
#   S = softmax_j(alpha masked by adj(+self loops))
#   out[i,hc] = sum_j S[i,j,h] * xr[j,hc] + bias
#
# Identities used on device:
#   leaky_relu(z) = 0.2*z + 0.8*relu(z)
#   alpha[i,j,h] = 0.2*sl[j,h] + 0.2*sr[i,h] + 0.8*sum_c att[h,c]*relu(xl[j,hc]+xr[i,hc])
# exp(0.2*sr[i,h]) cancels in the softmax; exp(0.2*sl[j,h]) (= esl) is folded
# multiplicatively into the aggregation operand; the output bias is folded
# into the aggregation operand too, via (num + bias*den)/den.  The adjacency
# mask (0/1, head-expanded, host-prepared) multiplies the exp'd scores on the
# vector engine.
#
# Per core: 256 dest rows = 2 ib x 4 supers x 32 rows.  Per super the 16
# dest-row pairs all accumulate into ONE [128, 1024] PSUM tile using 4
# stationary "variants" (att columns at local offset 8v) x 4 tile positions,
# so PSUM row r = 32q + 8v + 4d + h and dest-in-core = sup*32 + 8q + 2v + d
# comes out in natural order.  One exp per super-half writes fp16 scores which
# a DMA crossbar transpose scatters straight into the S^T aggregation layout
# (the last super uses PE transposes to shorten the tail).
#
# Sharding: 8 cores = (batch b in 0..1) x (4 blocks of 256 destination rows).

import numpy as np

B, N, F, H, C = 2, 1024, 128, 4, 16
HC = H * C
NCORES = 8
NI = 256          # destination rows per core
NSUP = 8          # supers of 16 pairs (32 dest rows) each
NF8 = 0           # pairs per super computed in fp8 (0/2/4); error ~5e-3/pair-pair

_CACHE = {}
LAST_RESULTS = None


def _build_program():
    import concourse.bass as bass
    import concourse.mybir as mybir
    import concourse.tile as tile
    from concourse import bacc

    f32 = mybir.dt.float32
    f16 = mybir.dt.float16
    f8 = mybir.dt.float8e4
    Alu = mybir.AluOpType
    Act = mybir.ActivationFunctionType

    nc = bacc.Bacc(
        "TRN2",
        target_bir_lowering=False,
        debug=False,
        enable_asserts=False,
        num_devices=NCORES,
    )

    # ---- DRAM I/O ----
    xbT16 = nc.dram_tensor("xbT16", [F, N], f16, kind="ExternalInput").ap()
    xisT16 = nc.dram_tensor("xisT16", [F, NI], f16, kind="ExternalInput").ap()
    adjx4 = nc.dram_tensor("adjx4", [128, NSUP * N], f16, kind="ExternalInput").ap()
    wl216 = nc.dram_tensor("wl216", [F, 128], f16, kind="ExternalInput").ap()
    wr16 = nc.dram_tensor("wr16", [F, HC], f16, kind="ExternalInput").ap()
    blp = nc.dram_tensor("blp", [128, 1], f32, kind="ExternalInput").ap()
    brp = nc.dram_tensor("brp", [HC, 1], f32, kind="ExternalInput").ap()
    attv = nc.dram_tensor("attv", [F, 128], f16, kind="ExternalInput").ap()
    attdr16 = nc.dram_tensor("attdr16", [F, 512], f16, kind="ExternalInput").ap()
    id16m = nc.dram_tensor("id16m", [128, 128], f16, kind="ExternalInput").ap()
    attbp = nc.dram_tensor("attbp", [HC, 16], f16, kind="ExternalInput").ap()
    brpb = nc.dram_tensor("brpb", [HC, 1], f32, kind="ExternalInput").ap()
    out = nc.dram_tensor("out", [NI, HC], f32, kind="ExternalOutput").ap()

    with tile.TileContext(nc) as tc:
        _body(tc, nc, mybir, f32, f16, f8, Alu, Act,
              xbT16, xisT16, adjx4, wl216, wr16, blp, brp, attv, attdr16, id16m, attbp,
              brpb, out)

    nc.compile()
    return nc


def _body(tc, nc, mybir, f32, f16, f8, Alu, Act,
          xbT16, xisT16, adjx4, wl216, wr16, blp, brp, attv, attdr16, id16m, attbp,
          brpb, out):
    from contextlib import ExitStack
    ctx = ExitStack()
    with ctx:
        consts = ctx.enter_context(tc.tile_pool(name="consts", bufs=1))
        work = ctx.enter_context(tc.tile_pool(name="work", bufs=1))
        rp_pool = ctx.enter_context(tc.tile_pool(name="rp", bufs=26))
        rp8_pool = ctx.enter_context(tc.tile_pool(name="rp8", bufs=5))
        sc_pool = ctx.enter_context(tc.tile_pool(name="sc", bufs=4))
        outp = ctx.enter_context(tc.tile_pool(name="outp", bufs=2))
        psg = ctx.enter_context(tc.tile_pool(name="psg", bufs=2, space="PSUM"))
        psb = ctx.enter_context(tc.tile_pool(name="psb", bufs=1, space="PSUM"))
        psa = ctx.enter_context(tc.tile_pool(name="psa", bufs=2, space="PSUM"))

        dma = nc.sync.dma_start
        dma2 = nc.scalar.dma_start      # Act HWDGE queue: output stores
        dmaT = nc.sync.dma_start_transpose

        # x^T arrives pre-transposed from the host, so startup is plain DMAs
        # on one queue, ordered by when the pipeline needs each tensor.
        xT = consts.tile([F, N], f16, tag="xT")       # [f, node]
        xisT = consts.tile([F, NI], f16, tag="xisT")  # [f, dest-slice node]
        wl2_t = consts.tile([F, 128], f16, tag="wl2")
        wr_t = consts.tile([F, HC], f16, tag="wr")
        blp2_t = consts.tile([128, 1], f32, tag="blp2")
        brpb_t = consts.tile([HC, 1], f32, tag="brpb")  # b_r + bias (xr_mod)
        brp_t = consts.tile([HC, 1], f32, tag="brp")
        attv_t = consts.tile([F, 128], f16, tag="attv")
        attdr_t = consts.tile([F, 512], f16, tag="attdr")
        att8_t = consts.tile([F, 512], f8, tag="att8")
        id16_t = consts.tile([128, 128], f16, tag="id16")
        attbp_t = consts.tile([HC, 16], f16, tag="attbp")
        adjx_t = consts.tile([128, NSUP * N], f16, tag="adjx")
        dma(xT[:, 0:512], xbT16[:, 0:512])
        dma(wl2_t[:], wl216)
        dma(xT[:, 512:N], xbT16[:, 512:N])
        dma(xisT[:], xisT16)
        dma(blp2_t[:], blp)
        dma(brp_t[:], brp)
        dma(attv_t[:], attv)
        dma(wr_t[:], wr16)
        dma(adjx_t[:], adjx4)
        dma(attbp_t[:], attbp)
        dma(brpb_t[:], brpb)
        dma(id16_t[:], id16m)
        if NF8:
            dma(attdr_t[:], attdr16)
            nc.vector.tensor_copy(att8_t[:], attdr_t[:])

        # ---------- projections ----------
        # xl2T: (x@W_l+b_l)^T stacked twice on partitions (for pair bias adds)
        xl2T = consts.tile([128, N], f16, tag="xl2T")
        xrT16 = consts.tile([HC, N], f16, tag="xrT16")   # (x@W_r+b_r)^T
        xrsT = consts.tile([HC, NI], f32, tag="xrsT")    # dest-row slice, f32
        pj = psg.tile([128, N], f32, tag="g", name="pj")
        for half in range(2):
            s = slice(half * 512, (half + 1) * 512)
            nc.tensor.matmul(pj[:, s], wl2_t[:], xT[:, s], start=True, stop=True)
        pj3 = psb.tile([HC, NI], f32, tag="b", name="pj3")
        nc.tensor.matmul(pj3[:], wr_t[:], xisT[:], start=True, stop=True)
        for half in range(2):
            s = slice(half * 512, (half + 1) * 512)
            nc.scalar.activation(xl2T[:, s], pj[:, s], Act.Identity,
                                 bias=blp2_t[:, 0:1], scale=1.0)
        nc.scalar.activation(xrsT[:], pj3[:], Act.Identity,
                             bias=brp_t[:, 0:1], scale=1.0)
        pj2 = psg.tile([HC, N], f32, tag="g", name="pj2")
        for half in range(2):
            s = slice(half * 512, (half + 1) * 512)
            nc.tensor.matmul(pj2[:, s], wr_t[:], xT[:, s], start=True, stop=True)
        nc.scalar.activation(xrT16[:], pj2[:], Act.Identity,
                             bias=brpb_t[:, 0:1], scale=1.0)

        # ---------- xrp: per-pair bias columns [xr[2p] ; xr[2p+1]] ----------
        xrp = consts.tile([128, 128], f32, tag="xrp")
        ev = xrsT[:].rearrange("p (a two) -> p a two", two=2)
        nc.vector.tensor_copy(xrp[0:HC, :], ev[:, :, 0])
        nc.vector.tensor_copy(xrp[HC:128, :], ev[:, :, 1])

        # ---------- xr_mod build: [j128, k, h, 0:16]=xr*esl, [..,16]=esl ----
        def build_xr_mod():
            # sl[h,j] = sum_hc att_blk[hc,h]*xl[hc,j]; esl = exp(0.2*sl)
            psl = psb.tile([16, N], f32, tag="b", name="psl")
            for half in range(2):
                s = slice(half * 512, (half + 1) * 512)
                nc.tensor.matmul(psl[:, s], attbp_t[:], xl2T[0:HC, s],
                                 start=True, stop=True)
            eslT = work.tile([16, N], f16, tag="eslT", name="eslT")
            nc.scalar.activation(eslT[:], psl[:], Act.Exp, scale=0.2)
            xr_nat = work.tile([128, 8 * HC], f16, tag="xrnat", name="xr_nat")
            esln = work.tile([128, 8 * 16], f16, tag="esln", name="esln")
            dmaT(xr_nat[:].rearrange("p (k c) -> p k c", k=8), xrT16[:])
            dmaT(esln[:].rearrange("p (k e) -> p k e", k=8), eslT[:])
            xmv = xr_mod[:].rearrange("p (k h e) -> p k h e", k=8, h=H)
            xnv = xr_nat[:].rearrange("p (k h c) -> p k h c", k=8, h=H)
            rep = esln[:].rearrange("p (k e) -> p k e", k=8)[:, :, 0:H]
            # broadcast esl over the 16 channels
            repb = esln[:].rearrange("p (k e one) -> p k e one", k=8, one=1)
            repb = repb[:, :, 0:H, :].broadcast_to([128, 8, H, C])
            nc.vector.tensor_tensor(xmv[:, :, :, 0:C], xnv, repb, Alu.mult)
            nc.vector.tensor_copy(xmv[:, :, :, C], rep)

        xr_mod = consts.tile([128, 8 * 68], f16, tag="xrmod")

        # ---------- main streaming loop ----------
        # st_t[ib]: S^T tiles, [j128, k*512 + s4*128 + r], r = PSUM row layout
        st_t = [consts.tile([128, 8 * 512], f16, tag=f"stt{ib}",
                            name=f"stt{ib}") for ib in range(2)]

        # ---------- aggregation ----------
        def aggregate(ib):
            out_f = outp.tile([128, HC], f32, tag="outf", name="outf")
            stv = st_t[ib][:].rearrange("p (k t h) -> p k t h", k=8, h=H)
            agg = psa.tile([128, 4 * 17], f32, tag="a", name="agg")
            for h in range(H):
                for k in range(8):
                    nc.tensor.matmul(agg[:, h * 17:(h + 1) * 17],
                                     stv[:, k, :, h],
                                     xr_mod[:, k * 68 + h * 17: k * 68 + (h + 1) * 17],
                                     start=(k == 0), stop=(k == 7))
            for h in range(H):
                rz = work.tile([128, 1], f32, tag="rz", name="rz")
                nc.vector.reciprocal(rz[:], agg[:, h * 17 + 16:h * 17 + 17])
                nc.vector.tensor_scalar(out_f[:, h * 16:(h + 1) * 16],
                                        agg[:, h * 17:h * 17 + 16], rz[:, 0:1],
                                        None, Alu.mult)
            dma2(out[ib * 128:(ib + 1) * 128, :], out_f[:])

        for sup in range(NSUP):
            ib, s4 = sup // 4, sup % 4
            if sup == 1:
                build_xr_mod()
            if sup == 4:
                aggregate(0)
            gps = psg.tile([128, N], f32, tag="g", name=f"gps{sup}")
            # fp8 slots (b,u): each pair is one DoubleRow matmul with
            # ktile0 = fp8(att), ktile1 = fp8 residual of att, both k-tiles
            # streaming the same rp8 (stride-0 AP).  DoubleRow only supports
            # tile position (0,0), so fp8 slots live in PSUM rows 0..64.
            f8slots = [(0, 3), (1, 3), (0, 2), (1, 2)][:NF8]
            rp8s = []
            for (b8, u8) in f8slots:
                rp8 = rp8_pool.tile([128, N], f8, tag="rp8")
                p = sup * 16 + b8 * 4 + u8
                nc.scalar.activation(rp8[:], xl2T[:], Act.Relu,
                                     bias=xrp[:, p:p + 1], scale=1.0)
                rp8s.append(rp8[:].rearrange("p (one j) -> p one j", one=1))
            rps = {}
            for q in range(4):
                for v in range(4):
                    if (q, v) in f8slots:
                        continue
                    p = sup * 16 + q * 4 + v
                    rp = rp_pool.tile([128, N], f16, tag="rp")
                    nc.vector.tensor_scalar(rp[:], xl2T[:], xrp[:, p:p + 1],
                                            0.0, Alu.add, Alu.max)
                    rps[q, v] = rp
            for q in range(4):
                for v in range(4):
                    if (q, v) in f8slots:
                        continue
                    for half in range(2):
                        s = slice(half * 512, (half + 1) * 512)
                        nc.tensor.matmul(
                            gps[32 * q:32 * q + 32, s],
                            attv_t[:, 32 * v:32 * v + 32],
                            rps[q, v][:, s],
                            start=(v == 0), stop=(v == 3),
                            tile_position=(0, 32 * q),
                            skip_group_check=True,
                        )
            for si in range(NF8):
                for half in range(2):
                    s = slice(half * 512, (half + 1) * 512)
                    nc.tensor.matmul(
                        gps[0:64, s],
                        att8_t[:, 128 * si:128 * si + 128].rearrange(
                            "p (t m) -> p t m", t=2),
                        rp8s[si][:, :, s].broadcast_to([128, 2, 512]),
                        start=False, stop=(si == NF8 - 1),
                        perf_mode=mybir.MatmulPerfMode.DoubleRow,
                        tile_position=(0, 0),
                        skip_group_check=True,
                    )
            scomp = sc_pool.tile([128, N], f16, tag="scomp")
            scm = sc_pool.tile([128, N], f16, tag="scm")
            dstv = st_t[ib][:].rearrange("p (k s r) -> p k s r",
                                         k=8, s=4)
            for half in range(2):
                s = slice(half * 512, (half + 1) * 512)
                nc.scalar.activation(scomp[:, s], gps[:, s], Act.Exp)
                # adjacency mask (0/1, head-expanded) applied on vector engine
                nc.vector.tensor_tensor(
                    scm[:, s], scomp[:, s],
                    adjx_t[:, sup * N + half * 512: sup * N + half * 512 + 512],
                    Alu.mult)
                if sup == NSUP - 1:
                    # tail: PE transpose (short latency) instead of DMA xbar
                    for k in range(half * 4, half * 4 + 4):
                        pt = psa.tile([128, 128], f16, tag="a", name="pt")
                        nc.tensor.transpose(pt[:], scm[:, k * 128:(k + 1) * 128],
                                            id16_t[:])
                        nc.vector.tensor_copy(dstv[:, k, s4, :], pt[:])
                else:
                    dmaT(dstv[:, half * 4:(half + 1) * 4, s4, :], scm[:, s])

        aggregate(1)


def _get_program():
    if "nc" not in _CACHE:
        _CACHE["nc"] = _build_program()
    return _CACHE["nc"]


def kernel(x, adj, W_l, b_l, W_r, b_r, att, bias):
    global LAST_RESULTS
    from concourse.bass_utils import run_bass_kernel_spmd

    x = np.ascontiguousarray(np.asarray(x, dtype=np.float32))
    adj = np.ascontiguousarray(np.asarray(adj, dtype=np.float32))
    W_l = np.asarray(W_l, dtype=np.float32)
    b_l = np.asarray(b_l, dtype=np.float32)
    W_r = np.asarray(W_r, dtype=np.float32)
    b_r = np.asarray(b_r, dtype=np.float32)
    att = np.asarray(att, dtype=np.float32)
    bias = np.asarray(bias, dtype=np.float32)

    # host-side constant prep
    attv = np.zeros((F, 128), np.float32)
    for v in range(4):
        for d in range(2):
            for h in range(H):
                col = 32 * v + 8 * v + 4 * d + h
                attv[d * HC + h * C:d * HC + (h + 1) * C, col] = 0.8 * att[h]
    attv = attv.astype(np.float16)
    import ml_dtypes
    attdr = np.zeros((F, 4, 2, 64), np.float32)
    for si, (b8, u8) in enumerate([(0, 3), (1, 3), (0, 2), (1, 2)]):
        for d in range(2):
            for h in range(H):
                m = 32 * b8 + 8 * u8 + 4 * d + h
                a = 0.8 * att[h]
                amain = a.astype(ml_dtypes.float8_e4m3).astype(np.float32)
                ares = (a - amain).astype(ml_dtypes.float8_e4m3).astype(np.float32)
                attdr[d * HC + h * C:d * HC + (h + 1) * C, si, 0, m] = amain
                attdr[d * HC + h * C:d * HC + (h + 1) * C, si, 1, m] = ares
    attdr16 = attdr.reshape(F, 512).astype(np.float16)
    id16 = np.eye(128, dtype=np.float16)
    # PSUM row r = 32b+8u+4d+h  <->  dest-in-super ld = 8b+2u+d
    rowld = np.zeros(128, np.int64)
    for b8 in range(4):
        for u8 in range(4):
            for d in range(2):
                for h in range(H):
                    rowld[32 * b8 + 8 * u8 + 4 * d + h] = 8 * b8 + 2 * u8 + d
    attbp = np.zeros((HC, 16), np.float32)
    for h in range(H):
        attbp[h * C:(h + 1) * C, h] = att[h]
    attbp = attbp.astype(np.float16)
    blp = np.concatenate([b_l, b_l]).reshape(128, 1).astype(np.float32)
    brp = b_r.reshape(HC, 1).astype(np.float32).copy()
    brpb = (b_r + bias).reshape(HC, 1).astype(np.float32).copy()
    wl216 = np.concatenate([W_l, W_l], axis=1).astype(np.float16)
    wr16 = W_r.astype(np.float16).copy()

    in_maps = []
    for core in range(NCORES):
        b, blk = core // 4, core % 4
        i0 = blk * NI
        adjsl = adj[b, i0:i0 + NI, :].copy()
        adjsl[np.arange(NI), i0 + np.arange(NI)] = 1.0   # self loops
        # adjx4[r, sup*N+j] = adj[sup*32 + rowld[r], j]  (head-expanded 0/1)
        a3 = adjsl.reshape(NSUP, 32, N)[:, rowld, :]
        adjx = np.ascontiguousarray(a3.transpose(1, 0, 2)).reshape(128, NSUP * N)
        adjx = adjx.astype(np.float16)
        in_maps.append({
            "xbT16": np.ascontiguousarray(x[b].T).astype(np.float16),
            "xisT16": np.ascontiguousarray(x[b, i0:i0 + NI].T).astype(np.float16),
            "adjx4": adjx, "id16m": id16,
            "wl216": wl216, "wr16": wr16, "blp": blp, "brp": brp,
            "attv": attv, "attdr16": attdr16, "attbp": attbp,
            "brpb": brpb,
        })

    nc = _get_program()
    res = run_bass_kernel_spmd(nc, in_maps, core_ids=list(range(NCORES)))
    LAST_RESULTS = res
    outp = np.zeros((B, N, HC), np.float32)
    for core in range(NCORES):
        b, blk = core // 4, core % 4
        outp[b, blk * NI:(blk + 1) * NI, :] = res.results[core]["out"]
    return outp



# revision 2
# speedup vs baseline: 1.2871x; 1.2871x over previous
# DenseGATv2Conv Trainium2 kernel (v3).
#
# Math (per batch b):
#   xl = x @ W_l + b_l ; xr = x @ W_r + b_r            [N, H*C]
#   alpha[i,j,h] = sum_c att[h,c] * leaky_relu(xl[j,hc] + xr[i,hc], 0.2)
#   S = softmax_j(alpha masked by adj(+self loops))
#   out[i,hc] = sum_j S[i,j,h] * xr[j,hc] + bias
#
# Identities used on device:
#   leaky_relu(z) = 0.2*z + 0.8*relu(z)
#   alpha[i,j,h] = 0.2*sl[j,h] + 0.2*sr[i,h] + 0.8*sum_c att[h,c]*relu(xl[j,hc]+xr[i,hc])
# exp(0.2*sr[i,h]) cancels in the softmax; exp(0.2*sl[j,h]) (= esl) is folded
# multiplicatively into the aggregation operand; the output bias is folded
# into the aggregation operand too, via (num + bias*den)/den.
#
# v3 changes vs v2:
#  * 9 of 16 pairs per super run in fp8: relu data is produced directly in
#    fp8e4m3 (DVE / Act / GpSimd share the production load) and consumed by
#    DoubleRow matmuls that pack TWO pairs per pass (2 k-tiles, disjoint
#    stationary columns), costing 0.5 PE cycles/row.  The fp8 rounding of
#    0.8*att is compensated exactly by scaling the relu production by
#    ratio[hc] = 0.8*att/fp8(0.8*att) (folded into the Act scale operand /
#    a prescaled copy of xl^T), so only the relu-value quantization noise
#    remains (~1.2e-2 rel).
#  * The adjacency mask is applied as a -15 additive bias inside the score
#    PSUM accumulation via one more fp8 DoubleRow matmul per half (moving =
#    -15*(1-adj) host-prepared fp8, stationary = 0/1 dest-row selector).
#    This removes the post-exp DVE multiply and 1.75MB of DMA.
#  * The remaining 7 pairs per super stay fp16 (DVE production + fp16
#    matmuls with tile-position banding) to keep the overall rel error
#    ~1.2e-2, under the 2e-2 gate.
#
# Sharding: 8 cores = (batch b in 0..1) x (4 blocks of 256 destination rows).

import numpy as np

B, N, F, H, C = 2, 1024, 128, 4, 16
HC = H * C
NCORES = 8
NI = 256          # destination rows per core
NSUP = 8          # supers of 16 pairs (32 dest rows) each

# fp8 duo passes: [(q,v),(q,v+1)] share one DoubleRow matmul per half.
FP8_DUOS = [((0, 0), (0, 1)), ((0, 2), (0, 3)),
            ((1, 0), (1, 1)), ((1, 2), (1, 3))]
FP8_SOLO = (2, 0)
F16_PAIRS = [(2, 1), (2, 2), (2, 3), (3, 0), (3, 1), (3, 2), (3, 3)]


def _fp8_engine(sup, q, v):
    # production engine per fp8 pair, balancing DVE/Act/Pool load
    if (q, v) in ((0, 0), (0, 1)):
        return "act"
    if (q, v) == (0, 2):
        return "act" if sup % 2 == 0 else "dve"
    if (q, v) in ((1, 0), (1, 1)):
        return "pool"
    if (q, v) == (1, 2):
        return "pool" if sup < 7 else "dve"
    return "dve"   # (0,3), (1,3), (2,0)


_CACHE = {}
LAST_RESULTS = None


def _build_program():
    import concourse.bass as bass
    import concourse.mybir as mybir
    import concourse.tile as tile
    from concourse import bacc

    f32 = mybir.dt.float32
    f16 = mybir.dt.float16
    f8 = mybir.dt.float8e4
    Alu = mybir.AluOpType
    Act = mybir.ActivationFunctionType

    nc = bacc.Bacc(
        "TRN2",
        target_bir_lowering=False,
        debug=False,
        enable_asserts=False,
        num_devices=NCORES,
    )

    # ---- DRAM I/O ----
    xbT16 = nc.dram_tensor("xbT16", [F, N], f16, kind="ExternalInput").ap()
    xisT16 = nc.dram_tensor("xisT16", [F, NI], f16, kind="ExternalInput").ap()
    wl216 = nc.dram_tensor("wl216", [F, 128], f16, kind="ExternalInput").ap()
    wr16 = nc.dram_tensor("wr16", [F, HC], f16, kind="ExternalInput").ap()
    blp = nc.dram_tensor("blp", [128, 1], f32, kind="ExternalInput").ap()
    brp = nc.dram_tensor("brp", [HC, 1], f32, kind="ExternalInput").ap()
    brpb = nc.dram_tensor("brpb", [HC, 1], f32, kind="ExternalInput").ap()
    attv = nc.dram_tensor("attv", [F, 128], f16, kind="ExternalInput").ap()
    a8stm = nc.dram_tensor("a8stm", [128, 1280], f8, kind="ExternalInput").ap()
    mskst = nc.dram_tensor("mskst", [16, 256], f8, kind="ExternalInput").ap()
    adjm8 = nc.dram_tensor("adjm8", [16, 16384], f8, kind="ExternalInput").ap()
    ratiop = nc.dram_tensor("ratiop", [128, 1], f32, kind="ExternalInput").ap()
    attbp = nc.dram_tensor("attbp", [HC, 16], f16, kind="ExternalInput").ap()
    id16m = nc.dram_tensor("id16m", [128, 128], f16, kind="ExternalInput").ap()
    out = nc.dram_tensor("out", [NI, HC], f32, kind="ExternalOutput").ap()

    with tile.TileContext(nc) as tc:
        _body(tc, nc, mybir, f32, f16, f8, Alu, Act,
              xbT16, xisT16, wl216, wr16, blp, brp, brpb, attv, a8stm, mskst,
              adjm8, ratiop, attbp, id16m, out)

    nc.compile()
    return nc


def _body(tc, nc, mybir, f32, f16, f8, Alu, Act,
          xbT16, xisT16, wl216, wr16, blp, brp, brpb, attv, a8stm, mskst,
          adjm8, ratiop, attbp, id16m, out):
    from contextlib import ExitStack
    ctx = ExitStack()
    with ctx:
        consts = ctx.enter_context(tc.tile_pool(name="consts", bufs=1))
        work = ctx.enter_context(tc.tile_pool(name="work", bufs=1))
        rp_pool = ctx.enter_context(tc.tile_pool(name="rp", bufs=16))
        duo_pool = ctx.enter_context(tc.tile_pool(name="duo", bufs=10))
        solo_pool = ctx.enter_context(tc.tile_pool(name="solo", bufs=3))
        sc_pool = ctx.enter_context(tc.tile_pool(name="sc", bufs=4))
        outp = ctx.enter_context(tc.tile_pool(name="outp", bufs=2))
        psg = ctx.enter_context(tc.tile_pool(name="psg", bufs=2, space="PSUM"))
        psb = ctx.enter_context(tc.tile_pool(name="psb", bufs=1, space="PSUM"))
        psa = ctx.enter_context(tc.tile_pool(name="psa", bufs=2, space="PSUM"))

        dma = nc.sync.dma_start
        dma2 = nc.scalar.dma_start      # Act HWDGE queue: output stores
        dmaT = nc.sync.dma_start_transpose

        xT = consts.tile([F, N], f16, tag="xT")       # [f, node]
        xisT = consts.tile([F, NI], f16, tag="xisT")  # [f, dest-slice node]
        wl2_t = consts.tile([F, 128], f16, tag="wl2")
        wr_t = consts.tile([F, HC], f16, tag="wr")
        blp2_t = consts.tile([128, 1], f32, tag="blp2")
        brp_t = consts.tile([HC, 1], f32, tag="brp")
        brpb_t = consts.tile([HC, 1], f32, tag="brpb")  # b_r + bias
        attv_t = consts.tile([F, 128], f16, tag="attv")
        a8st_t = consts.tile([128, 1280], f8, tag="a8st")
        mskst_t = consts.tile([16, 256], f8, tag="mskst")
        adjm_t = consts.tile([16, 16384], f8, tag="adjm")
        ratio_t = consts.tile([128, 1], f32, tag="ratio")
        attbp_t = consts.tile([HC, 16], f16, tag="attbp")
        id16_t = consts.tile([128, 128], f16, tag="id16")
        dma(xT[:, 0:512], xbT16[:, 0:512])
        dma(wl2_t[:], wl216)
        dma(blp2_t[:], blp)
        dma(ratio_t[:], ratiop)
        dma(xT[:, 512:N], xbT16[:, 512:N])
        dma(xisT[:], xisT16)
        dma(brp_t[:], brp)
        dma(wr_t[:], wr16)
        dma(a8st_t[:], a8stm)
        dma(attv_t[:], attv)
        dma(mskst_t[:], mskst)
        dma(adjm_t[:], adjm8)
        dma(attbp_t[:], attbp)
        dma(brpb_t[:], brpb)
        dma(id16_t[:], id16m)

        # ---------- projections ----------
        xl2T = consts.tile([128, N], f16, tag="xl2T")    # (x@W_l+b_l)^T x2
        xlh2T = consts.tile([128, N], f16, tag="xlh2T")  # xl2T * ratio
        xrT16 = consts.tile([HC, N], f16, tag="xrT16")   # (x@W_r+b_r)^T
        xrsT = consts.tile([HC, NI], f32, tag="xrsT")    # dest-row slice, f32
        pj = psg.tile([128, N], f32, tag="g", name="pj")
        for half in range(2):
            s = slice(half * 512, (half + 1) * 512)
            nc.tensor.matmul(pj[:, s], wl2_t[:], xT[:, s], start=True, stop=True)
        pj3 = psb.tile([HC, NI], f32, tag="b", name="pj3")
        nc.tensor.matmul(pj3[:], wr_t[:], xisT[:], start=True, stop=True)
        for half in range(2):
            s = slice(half * 512, (half + 1) * 512)
            nc.scalar.activation(xl2T[:, s], pj[:, s], Act.Identity,
                                 bias=blp2_t[:, 0:1], scale=1.0)
        nc.vector.tensor_scalar(xlh2T[:], xl2T[:], ratio_t[:, 0:1], 0.0,
                                Alu.mult, Alu.bypass)
        nc.scalar.activation(xrsT[:], pj3[:], Act.Identity,
                             bias=brp_t[:, 0:1], scale=1.0)
        pj2 = psg.tile([HC, N], f32, tag="g", name="pj2")
        for half in range(2):
            s = slice(half * 512, (half + 1) * 512)
            nc.tensor.matmul(pj2[:, s], wr_t[:], xT[:, s], start=True, stop=True)
        nc.scalar.activation(xrT16[:], pj2[:], Act.Identity,
                             bias=brpb_t[:, 0:1], scale=1.0)

        # ---------- xrp: per-pair bias columns [xr[2p] ; xr[2p+1]] ----------
        xrp = consts.tile([128, 128], f32, tag="xrp")
        xrph = consts.tile([128, 128], f32, tag="xrph")  # * ratio
        ev = xrsT[:].rearrange("p (a two) -> p a two", two=2)
        nc.vector.tensor_copy(xrp[0:HC, :], ev[:, :, 0])
        nc.vector.tensor_copy(xrp[HC:128, :], ev[:, :, 1])
        nc.vector.tensor_scalar(xrph[:], xrp[:], ratio_t[:, 0:1], 0.0,
                                Alu.mult, Alu.bypass)

        # ---------- xr_mod build: [j128, k, h, 0:16]=xr*esl, [..,16]=esl ----
        def build_xr_mod():
            psl = psb.tile([16, N], f32, tag="b", name="psl")
            for half in range(2):
                s = slice(half * 512, (half + 1) * 512)
                nc.tensor.matmul(psl[:, s], attbp_t[:], xl2T[0:HC, s],
                                 start=True, stop=True)
            eslT = work.tile([16, N], f16, tag="eslT", name="eslT")
            nc.scalar.activation(eslT[:], psl[:], Act.Exp, scale=0.2)
            xr_nat = work.tile([128, 8 * HC], f16, tag="xrnat", name="xr_nat")
            esln = work.tile([128, 8 * 16], f16, tag="esln", name="esln")
            dmaT(xr_nat[:].rearrange("p (k c) -> p k c", k=8), xrT16[:])
            dmaT(esln[:].rearrange("p (k e) -> p k e", k=8), eslT[:])
            xmv = xr_mod[:].rearrange("p (k h e) -> p k h e", k=8, h=H)
            xnv = xr_nat[:].rearrange("p (k h c) -> p k h c", k=8, h=H)
            rep = esln[:].rearrange("p (k e) -> p k e", k=8)[:, :, 0:H]
            repb = esln[:].rearrange("p (k e one) -> p k e one", k=8, one=1)
            repb = repb[:, :, 0:H, :].broadcast_to([128, 8, H, C])
            nc.vector.tensor_tensor(xmv[:, :, :, 0:C], xnv, repb, Alu.mult)
            nc.vector.tensor_copy(xmv[:, :, :, C], rep)

        xr_mod = consts.tile([128, 8 * 68], f16, tag="xrmod")

        # st_t[ib]: S^T tiles, [j128, k*512 + s4*128 + r], r = PSUM row layout
        st_t = [consts.tile([128, 8 * 512], f16, tag=f"stt{ib}",
                            name=f"stt{ib}") for ib in range(2)]

        # ---------- aggregation ----------
        def aggregate(ib):
            out_f = outp.tile([128, HC], f32, tag="outf", name="outf")
            stv = st_t[ib][:].rearrange("p (k t h) -> p k t h", k=8, h=H)
            agg = psa.tile([128, 4 * 17], f32, tag="a", name="agg")
            for h in range(H):
                for k in range(8):
                    nc.tensor.matmul(agg[:, h * 17:(h + 1) * 17],
                                     stv[:, k, :, h],
                                     xr_mod[:, k * 68 + h * 17: k * 68 + (h + 1) * 17],
                                     start=(k == 0), stop=(k == 7))
            for h in range(H):
                rz = work.tile([128, 1], f32, tag="rz", name="rz")
                nc.vector.reciprocal(rz[:], agg[:, h * 17 + 16:h * 17 + 17])
                nc.vector.tensor_scalar(out_f[:, h * 16:(h + 1) * 16],
                                        agg[:, h * 17:h * 17 + 16], rz[:, 0:1],
                                        None, Alu.mult)
            dma2(out[ib * 128:(ib + 1) * 128, :], out_f[:])

        a8v = a8st_t[:].rearrange("p (ps u c) -> p ps u c", ps=5, u=2)
        mskv = mskst_t[:].rearrange("p (u c) -> p u c", u=2)
        adjv = adjm_t[:].rearrange("p (u S j) -> p u S j", u=2, S=NSUP)

        for sup in range(NSUP):
            ib, s4 = sup // 4, sup % 4
            if sup == 1:
                build_xr_mod()
            if sup == 4:
                aggregate(0)
            gps = psg.tile([128, N], f32, tag="g", name=f"gps{sup}")

            # ---- fp8 production (9 pairs -> 4 duo tiles + 1 solo) ----
            duos = [duo_pool.tile([128, 2048], f8, tag="duo",
                                  name=f"duo{sup}_{j}") for j in range(4)]
            solo = solo_pool.tile([128, N], f8, tag="solo", name=f"solo{sup}")
            for j, (pa, pb) in enumerate(FP8_DUOS):
                for u, (q, v) in enumerate((pa, pb)):
                    p = sup * 16 + 4 * q + v
                    dst = duos[j][:, u * N:(u + 1) * N]
                    eng = _fp8_engine(sup, q, v)
                    if eng == "act":
                        nc.scalar.activation(dst, xl2T[:], Act.Relu,
                                             bias=xrph[:, p:p + 1],
                                             scale=ratio_t[:, 0:1])
                    elif eng == "pool":
                        nc.gpsimd.tensor_scalar(dst, xlh2T[:], xrph[:, p:p + 1],
                                                0.0, Alu.add, Alu.max)
                    else:
                        nc.vector.tensor_scalar(dst, xlh2T[:], xrph[:, p:p + 1],
                                                0.0, Alu.add, Alu.max)
            q, v = FP8_SOLO
            p = sup * 16 + 4 * q + v
            eng = _fp8_engine(sup, q, v)
            if eng == "act":
                nc.scalar.activation(solo[:], xl2T[:], Act.Relu,
                                     bias=xrph[:, p:p + 1], scale=ratio_t[:, 0:1])
            else:
                nc.vector.tensor_scalar(solo[:], xlh2T[:], xrph[:, p:p + 1],
                                        0.0, Alu.add, Alu.max)
            solo_v = solo[:].rearrange("p (one j) -> p one j", one=1)

            # ---- f16 production (7 pairs, DVE) ----
            rps = {}
            for (q, v) in F16_PAIRS:
                p = sup * 16 + 4 * q + v
                rp = rp_pool.tile([128, N], f16, tag="rp")
                nc.vector.tensor_scalar(rp[:], xl2T[:], xrp[:, p:p + 1],
                                        0.0, Alu.add, Alu.max)
                rps[q, v] = rp

            # ---- score matmuls ----
            for half in range(2):
                s = slice(half * 512, (half + 1) * 512)
                for j in range(4):
                    mv = duos[j][:].rearrange("p (u j) -> p u j", u=2)
                    nc.tensor.matmul(
                        gps[:, s], a8v[:, j, :, :], mv[:, :, s],
                        start=(j == 0), stop=False,
                        perf_mode=mybir.MatmulPerfMode.DoubleRow,
                        tile_position=(0, 0), skip_group_check=True)
                nc.tensor.matmul(
                    gps[:, s], a8v[:, 4, :, :],
                    solo_v[:, :, s].broadcast_to([128, 2, 512]),
                    start=False, stop=False,
                    perf_mode=mybir.MatmulPerfMode.DoubleRow,
                    tile_position=(0, 0), skip_group_check=True)
                nc.tensor.matmul(
                    gps[:, s], mskv[:, :, :], adjv[:, :, sup, s],
                    start=False, stop=False,
                    perf_mode=mybir.MatmulPerfMode.DoubleRow,
                    tile_position=(0, 0), skip_group_check=True)
                for (q, v) in F16_PAIRS:
                    nc.tensor.matmul(
                        gps[32 * q:32 * q + 32, s],
                        attv_t[:, 32 * v:32 * v + 32],
                        rps[q, v][:, s],
                        start=False, stop=((q, v) == F16_PAIRS[-1]),
                        tile_position=(0, 32 * q),
                        skip_group_check=True,
                    )

            # ---- exp + scatter to S^T layout ----
            dstv = st_t[ib][:].rearrange("p (k s r) -> p k s r", k=8, s=4)
            for half in range(2):
                s = slice(half * 512, (half + 1) * 512)
                scomp = sc_pool.tile([128, 512], f16, tag="scomp",
                                     name=f"sc{sup}_{half}")
                nc.scalar.activation(scomp[:], gps[:, s], Act.Exp)
                if sup == NSUP - 1:
                    # tail: PE transpose (short latency) instead of DMA xbar
                    for k in range(half * 4, half * 4 + 4):
                        pt = psa.tile([128, 128], f16, tag="a", name="pt")
                        nc.tensor.transpose(
                            pt[:], scomp[:, (k - half * 4) * 128:
                                         (k - half * 4 + 1) * 128], id16_t[:])
                        nc.vector.tensor_copy(dstv[:, k, s4, :], pt[:])
                else:
                    dmaT(dstv[:, half * 4:(half + 1) * 4, s4, :], scomp[:])

        aggregate(1)


def _get_program():
    if "nc" not in _CACHE:
        _CACHE["nc"] = _build_program()
    return _CACHE["nc"]


def kernel(x, adj, W_l, b_l, W_r, b_r, att, bias):
    global LAST_RESULTS
    import ml_dtypes
    from concourse.bass_utils import run_bass_kernel_spmd

    x = np.ascontiguousarray(np.asarray(x, dtype=np.float32))
    adj = np.ascontiguousarray(np.asarray(adj, dtype=np.float32))
    W_l = np.asarray(W_l, dtype=np.float32)
    b_l = np.asarray(b_l, dtype=np.float32)
    W_r = np.asarray(W_r, dtype=np.float32)
    b_r = np.asarray(b_r, dtype=np.float32)
    att = np.asarray(att, dtype=np.float32)
    bias = np.asarray(bias, dtype=np.float32)

    # ---- host-side constant prep ----
    # fp16 att stationary for the fp16 bands (q=2 v>=1, q=3)
    attv = np.zeros((F, 128), np.float32)
    for v in range(4):
        for d in range(2):
            for h in range(H):
                col = 32 * v + 8 * v + 4 * d + h
                attv[d * HC + h * C:d * HC + (h + 1) * C, col] = 0.8 * att[h]
    attv = attv.astype(np.float16)

    # fp8 att stationaries for the DoubleRow duo/solo passes.
    att8 = (0.8 * att.astype(np.float32)).astype(ml_dtypes.float8_e4m3)
    att8f = att8.astype(np.float32)
    # ratio[hc] = 0.8*att/fp8(0.8*att) (1.0 where att==0), dup'd over d
    with np.errstate(divide="ignore", invalid="ignore"):
        rat = np.where(att8f != 0.0, 0.8 * att / att8f, 1.0)
    ratio = np.concatenate([rat.reshape(HC), rat.reshape(HC)])
    ratio = ratio.reshape(128, 1).astype(np.float32)

    a8st = np.zeros((128, 5, 2, 128), np.float32)
    passes = list(FP8_DUOS) + [(FP8_SOLO, None)]
    for ps, (pa, pb) in enumerate(passes):
        for u, pair in enumerate((pa, pb)):
            if pair is None:
                continue
            q, v = pair
            for d in range(2):
                for h in range(H):
                    col = 32 * q + 8 * v + 4 * d + h
                    a8st[d * HC + h * C:d * HC + (h + 1) * C, ps, u, col] = att8f[h]
    a8stm = a8st.reshape(128, 1280).astype(ml_dtypes.float8_e4m3)

    # PSUM row r = 32q+8v+4d+h  <->  dest-in-super ld = 8q+2v+d
    rowld = np.zeros(128, np.int64)
    for q in range(4):
        for v in range(4):
            for d in range(2):
                for h in range(H):
                    rowld[32 * q + 8 * v + 4 * d + h] = 8 * q + 2 * v + d
    # mask stationary: mskst[ld%16, ld//16, r] = 1 where rowld[r]=ld
    mskst = np.zeros((16, 2, 128), np.float32)
    for r in range(128):
        ld = rowld[r]
        mskst[ld % 16, ld // 16, r] = 1.0
    mskst = mskst.reshape(16, 256).astype(ml_dtypes.float8_e4m3)

    attbp = np.zeros((HC, 16), np.float32)
    for h in range(H):
        attbp[h * C:(h + 1) * C, h] = att[h]
    attbp = attbp.astype(np.float16)
    blp = np.concatenate([b_l, b_l]).reshape(128, 1).astype(np.float32)
    brp = b_r.reshape(HC, 1).astype(np.float32).copy()
    brpb = (b_r + bias).reshape(HC, 1).astype(np.float32).copy()
    wl216 = np.concatenate([W_l, W_l], axis=1).astype(np.float16)
    wr16 = W_r.astype(np.float16).copy()
    id16 = np.eye(128, dtype=np.float16)

    in_maps = []
    for core in range(NCORES):
        b, blk = core // 4, core % 4
        i0 = blk * NI
        adjsl = adj[b, i0:i0 + NI, :].copy()
        adjsl[np.arange(NI), i0 + np.arange(NI)] = 1.0   # self loops
        # adjm8[k, u, sup, j] = -15*(1-adj[32*sup + k + 16u, j])
        a4 = adjsl.reshape(NSUP, 2, 16, N)   # [sup, u, k, j]
        adjm = -15.0 * (1.0 - a4.transpose(2, 1, 0, 3))   # [k, u, sup, j]
        adjm = np.ascontiguousarray(adjm).reshape(16, 16384)
        adjm = adjm.astype(ml_dtypes.float8_e4m3)
        in_maps.append({
            "xbT16": np.ascontiguousarray(x[b].T).astype(np.float16),
            "xisT16": np.ascontiguousarray(x[b, i0:i0 + NI].T).astype(np.float16),
            "wl216": wl216, "wr16": wr16, "blp": blp, "brp": brp,
            "brpb": brpb, "attv": attv, "a8stm": a8stm, "mskst": mskst,
            "adjm8": adjm, "ratiop": ratio, "attbp": attbp, "id16m": id16,
        })

    nc = _get_program()
    res = run_bass_kernel_spmd(nc, in_maps, core_ids=list(range(NCORES)))
    LAST_RESULTS = res
    outp = np.zeros((B, N, HC), np.float32)
    for core in range(NCORES):
        b, blk = core // 4, core % 4
        outp[b, blk * NI:(blk + 1) * NI, :] = res.results[core]["out"]
    return outp
